# revision 21
# baseline (speedup 1.0000x reference)
"""BFConv2d Trainium2 kernel.

Reference computation (per problem spec):
  xq = bfp_quantize(x)        # 8-bit mantissa, shared exp per 32 channels
  wq = bfp_quantize(weight)   # groups along Cin
  out = conv2d(xq, wq, stride 1, pad 1) + bias
  out = bfp_quantize(out)     # groups along Cout

Sharding: data-parallel over batch B=32 -> 4 images per core x 8 cores.
Weight/bias replicated; no collectives.

Per-core design:
  - BFP quantize (x, w, out): StreamTranspose puts the 32-channel groups on
    the free axis (group size == DVE 32x32 block transpose), reduce_max(abs)
    gives the group absmax, int32 bit tricks extract E = 2^floor(log2(amax))
    and 1/E.  The value path normalizes u = x/E on GPSIMD (exact power-of-2
    multiply), rounds via the fp32 magic constant T0 = 1.5*2^16 on the ACT
    engine (Identity with bias; RNE to the 1/128 grid == round-half-even),
    clips low via ACT Relu, clips high + un-offsets via one DVE 2-stage
    tensor_scalar (2x mode), rescales by E on GPSIMD, and StreamTransposes
    back.  All steps are bit-exact vs the jax reference quantizer.
  - Quantized x/w are exact in bf16 -> conv runs as bf16 matmuls
    (18 per psum tile: 2 ci-halves x 9 taps) accumulating fp32 in PSUM;
    x is stored zero-padded [(H+2)x(W+2)] so taps are AP shifts.
  - Output path: ACT copies PSUM->SBUF fused with the bias add, then the
    same quantize pipeline, then DMA out.
  - Work is software-pipelined: x-quant of image i+1 is emitted before
    conv of image i so DVE/GPSIMD/ACT/PE all stay busy.
"""

import os
import sys

sys.path.insert(0, "/opt/trn_rl_repo")

import numpy as np

import concourse.bass as bass
import concourse.mybir as mybir
import concourse.tile as tile
from concourse import bacc

F32 = mybir.dt.float32
I32 = mybir.dt.int32
BF16 = mybir.dt.bfloat16
AX = mybir.AxisListType
OP = mybir.AluOpType

N_CORES = 8
DIRECT_ST = True    # StreamTranspose directly into padded tile (HW only;
                    # the CoreSim transpose helper needs 2-D views)
B, CIN, H, W = 32, 256, 56, 56
COUT, KK = 256, 3
B_LOCAL = B // N_CORES
P = 128


# Quantize-in-normalized-domain constants.  After u = x * 2^-e (exact),
# round-to-1/128-grid via magic add T0 = 1.5*2^16; clip to [-1, 127/128].
T0 = 98304.0            # 1.5 * 2^16
LO_T0 = 98303.0         # T0 - 1
HI_T0 = 98304.9921875   # T0 + 127/128  (exact in fp32)


def _emit_quant_group(nc, pool, chunks, tag, dst_dtype, consts):
    ct_inv, bias_T0, bias_nLO = consts
    """BFP-quantize a half-image: chunks = [(src_ap [128, npix], dst_ap)].
    Groups = 32 consecutive partitions (block-transposed via StreamTranspose).

    Per chunk: ST -> absmax-reduce -> (consolidated exponent ops) ->
    u = x*2^-e [gpsimd] -> ts(+T0, max LO_T0) -> ts(min HI_T0, -T0) [DVE 2x]
    -> q = u*2^e [gpsimd, casts to dst dtype] -> ST-back.
    ct_inv: const int32 tile [128,1] holding (254<<23) for inverse exponent.
    """
    njs = [src.shape[-1] // 32 for src, _ in chunks]
    nj_tot = sum(njs)
    A = pool.tile([P, nj_tot], F32, tag=tag + "A", bufs=3)
    x_bts = []
    off = 0
    for (src, _), nj in zip(chunks, njs):
        npix = nj * 32
        x_bt = pool.tile([P, npix], F32, tag=tag + "b", name="x_bt",
                         bufs=min(len(chunks) + 2, 4))
        nc.vector.transpose(x_bt[:], src)
        nc.vector.tensor_reduce(
            A[:, off:off + nj],
            x_bt[:].rearrange("p (j f) -> p j f", f=32),
            axis=AX.X, op=OP.max, apply_absolute_value=True,
        )
        x_bts.append(x_bt)
        off += nj

    # E = 2^floor(log2(absmax)) (guarded >= 2^-126); invE = 2^-floor(...)
    Ai = A[:].bitcast(I32)
    nc.vector.tensor_single_scalar(Ai, Ai, 0x7F800000, OP.bitwise_and)
    invE = pool.tile([P, nj_tot], F32, tag=tag + "I", bufs=3)
    nc.vector.tensor_tensor(
        invE[:].bitcast(I32),
        ct_inv[:, 0:1].to_broadcast((P, nj_tot)), Ai,
        OP.subtract)

    off = 0
    for (src, dst), nj, x_bt in zip(chunks, njs, x_bts):
        npix = nj * 32

        def bcs(t):
            return t[:, off:off + nj, None].to_broadcast((P, nj, 32))

        def v3(t):
            return t[:].rearrange("p (j f) -> p j f", f=32)

        u = pool.tile([P, npix], F32, tag=tag + "w", name="u", bufs=4)
        nc.gpsimd.tensor_tensor(v3(u), v3(x_bt), bcs(invE), OP.mult)
        # v = RNE(u + T0): the magic round (ACT Identity bias-add, exact)
        y = pool.tile([P, npix], F32, tag=tag + "w", name="y", bufs=4)
        nc.scalar.activation(y[:], u[:],
                             mybir.ActivationFunctionType.Identity,
                             bias=bias_T0[:, 0:1])
        # w1 = max(v - LO_T0, 0)  (clip low; exact Sterbenz subtract)
        nc.scalar.activation(y[:], y[:],
                             mybir.ActivationFunctionType.Relu,
                             bias=bias_nLO[:, 0:1])
        # z = min(w1, 255/128) - 1  (clip high + undo offset)
        z = pool.tile([P, npix], F32, tag=tag + "w", name="z", bufs=4)
        nc.vector.tensor_scalar(z[:], y[:], 1.9921875, 1.0,
                                OP.min, OP.subtract)
        q = pool.tile([P, npix], dst_dtype, tag=tag + "q", name="q", bufs=3)
        nc.gpsimd.tensor_tensor(v3(q), v3(z), bcs(A), OP.mult)
        if len(dst.shape) == 3:
            nc.vector.transpose(
                dst, q[:].rearrange("p (r c) -> p r c", r=dst.shape[1]))
        else:
            nc.vector.transpose(dst, q[:])
        off += nj


def build_kernel(b_local=B_LOCAL, h=H, w=W):
    """Build the per-core Bass module."""
    nc = bacc.Bacc("TRN2")
    hw = h * w
    hp, wp = h + 2, w + 2
    hwp = hp * wp

    x_in = nc.dram_tensor("x", [b_local, CIN, h, w], F32, kind="ExternalInput")
    w_in = nc.dram_tensor("weight", [COUT, CIN, KK, KK], F32,
                          kind="ExternalInput")
    b_in = nc.dram_tensor("bias", [COUT], F32, kind="ExternalInput")
    o_out = nc.dram_tensor("out", [b_local, COUT, h, w], F32,
                           kind="ExternalOutput")

    # conv output tiling: rows per psum tile (free <= 448)
    rpt = max(1, 448 // w)
    while h % rpt:
        rpt -= 1
    n_pt = h // rpt
    npix_t = rpt * w

    # x-quant chunking: rows per chunk, rows*w % 32 == 0, ~<=800 px
    xc_rows = max(1, 784 // w)
    while h % xc_rows or (xc_rows * w) % 32:
        xc_rows -= 1
    n_xc = h // xc_rows
    xc_pix = xc_rows * w

    # out-quant chunking: whole psum tiles, pix % 32 == 0
    out_chunks = []  # (start_pt, pix)
    acc, start = 0, 0
    for pt in range(n_pt):
        acc += npix_t
        if acc % 32 == 0 and (acc >= 896 or pt == n_pt - 1):
            out_chunks.append((start, acc))
            start, acc = pt + 1, 0
    assert acc == 0, "out chunking failed"

    with tile.TileContext(nc) as tc:
        with tc.tile_pool(name="persist", bufs=1) as pp:
            ident = pp.tile([P, P], BF16, tag="ident")
            from concourse.masks import make_identity
            make_identity(nc, ident[:])

            ct_inv = pp.tile([P, 1], I32, tag="ctinv")
            nc.gpsimd.memset(ct_inv[:], 254 << 23)
            bias_T0 = pp.tile([P, 1], F32, tag="biasT0")
            nc.gpsimd.memset(bias_T0[:], T0)
            bias_nLO = pp.tile([P, 1], F32, tag="biasnLO")
            nc.gpsimd.memset(bias_nLO[:], -LO_T0)
            consts = (ct_inv, bias_T0, bias_nLO)

            bias_sb = pp.tile([P, 2], F32, tag="bias")
            for ch in range(2):
                nc.sync.dma_start(bias_sb[:, ch:ch + 1],
                                  b_in[ch * P:(ch + 1) * P, None])

            # lhsT[ci_half]: [128 ci, 9*256] bf16, free idx = khw*256 + co
            lhsT = [pp.tile([P, 9 * COUT], BF16, tag=f"lhsT{i}",
                            name=f"lhsT{i}") for i in range(2)]

            # ---------------- weight prep ----------------
            ng = CIN // 32
            with tc.tile_pool(name="wstart", bufs=1) as wsp, \
                 tc.tile_pool(name="wpsum", bufs=2, space="PSUM") as wpp:
                for co_half in range(2):
                    w_nat = wsp.tile([P, CIN * 9], F32, tag="wnat")
                    nc.sync.dma_start(
                        w_nat[:],
                        w_in[co_half * P:(co_half + 1) * P].rearrange(
                            "o i kh kw -> o (i kh kw)"))

                    A = wsp.tile([P, ng * 9], F32, tag="wA")
                    nc.vector.tensor_reduce(
                        A[:].rearrange("p (g k) -> p g k", g=ng),
                        w_nat[:].rearrange("p (g c k) -> p g k c", g=ng, c=32),
                        axis=AX.X, op=OP.max, apply_absolute_value=True,
                    )
                    Ai = A[:].bitcast(I32)
                    nc.vector.tensor_single_scalar(Ai, Ai, 0x7F800000,
                                                   OP.bitwise_and)
                    nc.vector.tensor_single_scalar(Ai, Ai, 0x00800000, OP.max)
                    invE = wsp.tile([P, ng * 9], F32, tag="wI")
                    nc.vector.tensor_tensor(
                        invE[:].bitcast(I32),
                        ct_inv[:, 0:1].to_broadcast((P, ng * 9)), Ai,
                        OP.subtract)

                    def wv(t):
                        return t[:].rearrange("p (g c k) -> p g c k",
                                              g=ng, c=32)

                    def wb(t):
                        return t[:].rearrange("p (g k) -> p g k", g=ng)[
                            :, :, None, :].to_broadcast((P, ng, 32, 9))

                    y1 = wsp.tile([P, CIN * 9], F32, tag="wy1")
                    nc.vector.tensor_tensor(wv(y1), wv(w_nat), wb(invE),
                                            OP.mult)
                    y2 = wsp.tile([P, CIN * 9], F32, tag="wy2")
                    nc.vector.tensor_scalar(y2[:], y1[:], T0, LO_T0,
                                            OP.add, OP.max)
                    y3 = wsp.tile([P, CIN * 9], F32, tag="wy1")
                    nc.vector.tensor_scalar(y3[:], y2[:], HI_T0, T0,
                                            OP.min, OP.subtract)
                    wq = wsp.tile([P, CIN * 9], BF16, tag="wq")
                    nc.gpsimd.tensor_tensor(wv(wq), wv(y3), wb(A), OP.mult)

                    # [co 128, ci 128 (stride 9)] -> [ci, co] per (ci_half,khw)
                    wq_kci = wq[:].rearrange("p (ci k) -> p k ci", k=9)
                    for ci_half in range(2):
                        for khw in range(9):
                            tp = wpp.tile([P, P], BF16, tag="wtp")
                            src = wq_kci[:, khw,
                                         ci_half * P:(ci_half + 1) * P]
                            nc.tensor.transpose(tp[:], src, ident[:])
                            nc.scalar.copy(
                                lhsT[ci_half][:, khw * COUT + co_half * P:
                                              khw * COUT + co_half * P + P],
                                tp[:])

            # ---------------- main pipeline ----------------
            with tc.tile_pool(name="xq", bufs=3) as xqp, \
                 tc.tile_pool(name="xs", bufs=2) as xsp, \
                 tc.tile_pool(name="xw", bufs=3) as xwp, \
                 tc.tile_pool(name="os", bufs=3) as osp, \
                 tc.tile_pool(name="ow", bufs=3) as owp, \
                 tc.tile_pool(name="cpsum", bufs=8, space="PSUM") as cpp:

                # x is DMA'd into an already zero-PADDED fp32 layout; the
                # quant pipeline then runs on contiguous FLAT chunks of the
                # padded image (pad pixels quantize to 0 harmlessly: their
                # group absmax is 0 -> q = 0).  This removes every strided
                # 3-D ST/reduce access (they measured ~20-30% slower) and
                # the qpad border memsets.
                hwq = 3456  # 4 uniform chunks of 864 (27 j-blocks each)
                xcq = hwq // 4
                assert hwq >= hwp and xcq % 32 == 0

                def emit_xquant(img):
                    xq_pad = []
                    for ci_half in range(2):
                        x_pad = xsp.tile([P, hwq], F32, tag="xnat",
                                         name="x_pad")
                        xpv = x_pad[:, 0:hwp].rearrange("p (r c) -> p r c",
                                                        r=hp)
                        nc.gpsimd.memset(xpv[:, 0:hp:hp - 1, :], 0.0)
                        nc.gpsimd.memset(xpv[:, :, 0:wp:wp - 1], 0.0)
                        nc.gpsimd.memset(x_pad[:, hwp:hwq], 0.0)
                        xv = x_in[img,
                                  ci_half * P:(ci_half + 1) * P].rearrange(
                            "c h w -> c h w")
                        for hh in range(0, h, 28):
                            nc.sync.dma_start(
                                xpv[:, 1 + hh:1 + hh + 28, 1:1 + w],
                                xv[:, hh:hh + 28, :])
                        qpad = xqp.tile([P, hwq], BF16, tag=f"qpad{ci_half}",
                                        name="qpad", bufs=2)
                        chunks = []
                        for xc in range(4):
                            chunks.append(
                                (x_pad[:, xc * xcq:(xc + 1) * xcq],
                                 qpad[:, xc * xcq:(xc + 1) * xcq]))
                        _emit_quant_group(nc, xwp, chunks, "x", BF16, consts)
                        xq_pad.append(qpad)
                    return xq_pad

                def emit_conv_out(img, xq_pad):
                    pt_groups = []
                    s = 0
                    while s < n_pt:
                        g = min(4, n_pt - s)
                        pt_groups.append(range(s, s + g))
                        s += g
                    for co_half in range(2):
                        o_nat = osp.tile([P, hw], F32, tag="onat",
                                         name="o_nat")
                        for pts in pt_groups:
                            pss = {pt: cpp.tile([P, npix_t], F32, tag="cps",
                                                name="ps")
                                   for pt in pts}
                            k = 0
                            for ci_half in range(2):
                                qpv = xq_pad[ci_half][:, 0:hwp].rearrange(
                                    "p (r c) -> p r c", r=hp)
                                for kh in range(3):
                                    for kw in range(3):
                                        khw = kh * 3 + kw
                                        for pt in pts:
                                            rhs = qpv[:, pt * rpt + kh:
                                                      pt * rpt + kh + rpt,
                                                      kw:kw + w]
                                            nc.tensor.matmul(
                                                pss[pt][:].rearrange(
                                                    "p (r c) -> p r c",
                                                    r=rpt),
                                                lhsT[ci_half][
                                                    :, khw * COUT +
                                                    co_half * P:
                                                    khw * COUT +
                                                    co_half * P + P],
                                                rhs,
                                                start=(k == 0),
                                                stop=(k == 17))
                                        k += 1
                            for pt in pts:
                                nc.scalar.activation(
                                    o_nat[:, pt * npix_t:(pt + 1) * npix_t],
                                    pss[pt][:],
                                    mybir.ActivationFunctionType.Identity,
                                    bias=bias_sb[:, co_half:co_half + 1])

                        chunks, oqs = [], []
                        for (spt, cpix) in out_chunks:
                            oq = osp.tile([P, cpix], F32, tag="oq",
                                          name="oq")
                            chunks.append(
                                (o_nat[:, spt * npix_t:spt * npix_t + cpix],
                                 oq[:]))
                            oqs.append(oq)
                        _emit_quant_group(nc, owp, chunks, "o", F32, consts)
                        for (spt, cpix), oq in zip(out_chunks, oqs):
                            nc.sync.dma_start(
                                o_out[img, co_half * P:
                                      (co_half + 1) * P].rearrange(
                                    "c h w -> c (h w)")[
                                    :, spt * npix_t:spt * npix_t + cpix],
                                oq[:])

                # software-pipelined emission: quant(img+1) before conv(img)
                prev = None
                for img in range(b_local):
                    cur = emit_xquant(img)
                    if prev is not None:
                        emit_conv_out(img - 1, prev)
                    prev = cur
                emit_conv_out(b_local - 1, prev)
    nc.compile()
    return nc


_NC_CACHE = {}


def _get_nc(key):
    if key not in _NC_CACHE:
        _NC_CACHE[key] = build_kernel(*key)
    return _NC_CACHE[key]


def kernel(x, weight, bias):
    from concourse import bass_utils

    nc = _get_nc((B_LOCAL, H, W))
    in_maps = []
    for core in range(N_CORES):
        in_maps.append({
            "x": np.ascontiguousarray(x[core * B_LOCAL:(core + 1) * B_LOCAL]),
            "weight": np.ascontiguousarray(weight),
            "bias": np.ascontiguousarray(bias),
        })
    res = bass_utils.run_bass_kernel_spmd(
        nc, in_maps, core_ids=list(range(N_CORES)),
        trace=bool(int(os.environ.get("BFC_TRACE", "0"))),
    )
    out = np.concatenate([r["out"] for r in res.results], axis=0)
    kernel.last_exec_time_ns = res.exec_time_ns
    kernel.last_mean_exec_time_ns = res.mean_exec_time_ns
    kernel.last_trace = res.instructions_and_trace
    return out


kernel.last_exec_time_ns = None

